# revision 26
# baseline (speedup 1.0000x reference)
"""AttentionPooling (segment softmax + weighted segment-sum) Trainium2 kernel.

Algorithm (reference without explicit seg_max subtraction — scores are tiny,
|s| < ~3, so exp() is numerically safe unshifted and softmax is
shift-invariant):

    s_i   = W2^T lrelu(W1^T x_i + b1) + b2          (per node)
    e_i   = exp(s_i)
    out_g = (sum_{i in g} e_i x_i) / (sum_{i in g} e_i + 1e-16)

Sharding: 16384 segments -> 8 cores x 16 blocks x 128 segments. batch is
sorted, so each (core, block) owns a contiguous node range. Each block is
further split into 4 sub-blocks of 32 segments; the host pads every
sub-block to the same number T32 of 128-node tiles (SPMD-uniform across
cores), so every tile's nodes fall inside one 32-segment window whose index
is a compile-time constant.

Software pipeline over 8-tile groups, 4 stages staggered one group apart so
every cross-engine dependency has >= 1 full group of slack:
  A(g):   DMA xT (fp8, score path) + xn (bf16 [ones | x | onehot0] per
          tile); mm1 (stacked lrelu trick, signs folded into
          w1c = [W1 | -W1]); relu split by columns: half ACT (bias=b1c),
          half DVE (tensor_scalar add+max).
  B(g-1): per-tile score matmul (w2c rhs) -> [128,8] psum column tile.
  B2(g-2): ONE ACT Exp (bias=b2) -> e-columns f32; ONE DVE tensor_tensor
          multiplies all 8 one-hots by their e-columns (broadcast AP).
  C(g-3): M=32 col-tiled pooling matmul accumulates [sum e | sum e*x] into
          the block's [128,129] PSUM at partition base 32*window; flush per
          block: +1e-16, reciprocal, scale, DMA out.
"""

import os
import numpy as np
import ml_dtypes

N = 2_000_000
D = 128
H = 64
G = 16384
NEG_SLOPE = 0.01
NCORES = 8
SEGS_PER_CORE = G // NCORES          # 2048
SEGS_PER_BLOCK = 128
BLOCKS_PER_CORE = SEGS_PER_CORE // SEGS_PER_BLOCK   # 16
SUBS_PER_BLOCK = 4                   # 32-segment windows
SEGW = 32
TILE_N = 128
GROUP = 8                            # tiles per group
CW = D + 1 + SEGW                    # tile width: [ones | x | onehot]
ACT_COLS = 512                       # relu columns on ACT; rest on DVE

_bf16 = ml_dtypes.bfloat16
_fp8 = ml_dtypes.float8_e4m3


def _host_prep(x, batch, W1, b1, W2, b2):
    bounds = np.searchsorted(batch, np.arange(0, G + 1, SEGW))
    cnts = np.diff(bounds)                       # nodes per 32-seg sub-block
    T32 = int(np.max((cnts + TILE_N - 1) // TILE_N))   # tiles per sub-block
    K = SUBS_PER_BLOCK * T32                     # tiles per block
    NT = BLOCKS_PER_CORE * K                     # tiles per core
    NG = (NT + GROUP - 1) // GROUP
    NTP = NG * GROUP                             # padded tile count

    iota = np.arange(SEGW, dtype=np.float32)
    in_maps = []
    for c in range(NCORES):
        xn = np.zeros((NTP, TILE_N, CW), dtype=np.float32)  # [tile, node, 1+D+32]
        for b in range(BLOCKS_PER_CORE):
            for q in range(SUBS_PER_BLOCK):
                sb = (c * BLOCKS_PER_CORE + b) * SUBS_PER_BLOCK + q
                lo, hi = int(bounds[sb]), int(bounds[sb + 1])
                n = hi - lo
                tb = b * K + q * T32
                blk = np.zeros((T32 * TILE_N, D), dtype=np.float32)
                blk[:n] = x[lo:hi]
                bl = np.full((T32 * TILE_N,), -1.0, dtype=np.float32)
                bl[:n] = (batch[lo:hi] - sb * SEGW).astype(np.float32)
                oh = (bl[:, None] == iota[None, :]).astype(np.float32)
                xn[tb:tb + T32, :, 1:1 + D] = blk.reshape(T32, TILE_N, D)
                xn[tb:tb + T32, :, 0] = 1.0
                xn[tb:tb + T32, :, 1 + D:] = oh.reshape(T32, TILE_N, SEGW)
        xn16 = xn.astype(_bf16)
        # xT groups (fp8): [NG*D, GROUP*TILE_N]
        xT = np.ascontiguousarray(
            xn[:, :, 1:1 + D].transpose(0, 2, 1)
            .reshape(NG, GROUP, D, TILE_N).transpose(0, 2, 1, 3)
            .reshape(NG * D, GROUP * TILE_N)).astype(_fp8)
        # xn groups: [NG*TILE_N, GROUP*CW]
        xng = np.ascontiguousarray(
            xn16.reshape(NG, GROUP, TILE_N, CW).transpose(0, 2, 1, 3)
            .reshape(NG * TILE_N, GROUP * CW))
        in_maps.append({"xT": xT, "xn": xng})

    consts = {
        "w1": np.ascontiguousarray(W1.astype(_fp8)),                       # [128, 64]
        "w2c": np.ascontiguousarray(
            np.concatenate([W2, W2], axis=0).astype(_bf16)),               # [128, 1]
        "b1c": np.ascontiguousarray(
            np.concatenate([b1, b1])[:, None].astype(np.float32)),         # [128, 1]
        "b2c": np.full((TILE_N, 1), float(b2[0]), dtype=np.float32),       # [128, 1]
    }
    for m in in_maps:
        m.update(consts)
    return in_maps, T32, float(b2[0])


def _build(T32, b2f):
    import concourse.bass as bass
    import concourse.bacc as bacc
    import concourse.mybir as mybir
    from concourse.tile import TileContext

    dt = mybir.dt
    f32, bf16, fp8 = dt.float32, dt.bfloat16, dt.float8e4
    Alu = mybir.AluOpType
    Act = mybir.ActivationFunctionType

    K = SUBS_PER_BLOCK * T32
    NT = BLOCKS_PER_CORE * K
    NG = (NT + GROUP - 1) // GROUP
    NTP = NG * GROUP

    nc = bacc.Bacc("TRN2", target_bir_lowering=False)
    xT_d = nc.dram_tensor("xT", [NG * D, GROUP * TILE_N], fp8, kind="ExternalInput")
    xn_d = nc.dram_tensor("xn", [NG * TILE_N, GROUP * CW], bf16, kind="ExternalInput")
    w1_d = nc.dram_tensor("w1", [D, H], fp8, kind="ExternalInput")
    w2_d = nc.dram_tensor("w2c", [TILE_N, 1], bf16, kind="ExternalInput")
    b1_d = nc.dram_tensor("b1c", [TILE_N, 1], f32, kind="ExternalInput")
    b2_d = nc.dram_tensor("b2c", [TILE_N, 1], f32, kind="ExternalInput")
    out_d = nc.dram_tensor("out", [SEGS_PER_CORE, D], f32, kind="ExternalOutput")

    xT_v = xT_d[:].rearrange("(g p) c -> g p c", p=D)
    xn_v = xn_d[:].rearrange("(g p) c -> g p c", p=TILE_N)

    with TileContext(nc) as tc:
        import contextlib
        ctx = contextlib.ExitStack()
        with ctx:
            cpool = ctx.enter_context(tc.tile_pool(name="consts", bufs=1))
            w1_s = cpool.tile([D, H], fp8, tag="w1")
            w2_s = cpool.tile([TILE_N, 1], bf16, tag="w2")
            b1_s = cpool.tile([TILE_N, 1], f32, tag="b1")
            b2_s = cpool.tile([TILE_N, 1], f32, tag="b2")
            nc.sync.dma_start(w1_s[:], w1_d[:])
            nc.sync.dma_start(w2_s[:], w2_d[:])
            nc.sync.dma_start(b1_s[:], b1_d[:])
            nc.sync.dma_start(b2_s[:], b2_d[:])

            xg_pool = ctx.enter_context(tc.tile_pool(name="xg", bufs=8))
            xn_pool = ctx.enter_context(tc.tile_pool(name="xnp", bufs=8))
            hsb_pool = ctx.enter_context(tc.tile_pool(name="hsb", bufs=4))
            ec_pool = ctx.enter_context(tc.tile_pool(name="ec", bufs=3))
            oh_pool = ctx.enter_context(tc.tile_pool(name="oh", bufs=3))
            ob_pool = ctx.enter_context(tc.tile_pool(name="ob", bufs=2))
            dn_pool = ctx.enter_context(tc.tile_pool(name="dn", bufs=2))

            hps_pool = ctx.enter_context(tc.tile_pool(name="hps", bufs=2, space="PSUM"))
            sps_pool = ctx.enter_context(tc.tile_pool(name="sps", bufs=2, space="PSUM"))
            pps_pool = ctx.enter_context(tc.tile_pool(name="pps", bufs=2, space="PSUM"))

            def flush(bt, pps):
                dn = dn_pool.tile([TILE_N, 1], f32, tag="dn")
                nc.vector.tensor_scalar(dn[:], pps[:, 0:1], 1e-16, None, op0=Alu.add)
                rc = dn_pool.tile([TILE_N, 1], f32, tag="rc")
                nc.vector.reciprocal(rc[:], dn[:])
                ob = ob_pool.tile([TILE_N, D], f32, tag="ob")
                nc.vector.tensor_scalar(ob[:], pps[:, 1:129], rc[:], None, op0=Alu.mult)
                nc.sync.dma_start(out_d[bt * TILE_N:(bt + 1) * TILE_N, :], ob[:])

            repeat = int(os.environ.get("BASSK_REPEAT", "1"))
            if repeat > 1:
                rloop = ctx.enter_context(tc.For_i(0, repeat, 1))

            xns = [None] * NG
            hsbs = [None] * NG
            spss = [None] * NG
            ohws = [None] * NG
            pps = None

            def stage_a(g):
                xg = xg_pool.tile([D, GROUP * TILE_N], fp8, tag="xg")
                nc.sync.dma_start(xg[:], xT_v[g, :, :])
                xn = xn_pool.tile([TILE_N, GROUP * CW], bf16, tag="xn")
                nc.sync.dma_start(xn[:], xn_v[g, :, :])
                xns[g] = xn
                # H=64 MLP: two node-halves packed into one [128, 512] PSUM
                # bank at col-strips 0 and 64.
                hps = hps_pool.tile([TILE_N, 512], f32, tag="hps")
                for half in range(2):
                    nc.tensor.matmul(
                        hps[half * H:(half + 1) * H, :],
                        w1_s[:],
                        xg[:, half * 512:(half + 1) * 512],
                        start=True, stop=True,
                        tile_position=(0, half * H),
                        skip_group_check=True,
                    )
                hsb = hsb_pool.tile([TILE_N, 512], bf16, tag="hsb")
                nc.scalar.activation(hsb[:], hps[:], Act.Prelu,
                                     bias=b1_s[:], scale=1.0, alpha=NEG_SLOPE)
                hsbs[g] = hsb

            def stage_b(g):
                hsb = hsbs[g]
                sps = sps_pool.tile([TILE_N, GROUP], f32, tag="sps")
                # alternate row-halves so consecutive LDWEIGHTS hit different
                # row groups and pipeline with in-flight matmuls
                for j in range(GROUP):
                    hb = (j // 4) * H
                    jj = j % 4
                    nc.tensor.matmul(
                        sps[:, j:j + 1],
                        hsb[hb:hb + H, jj * TILE_N:(jj + 1) * TILE_N],
                        w2_s[hb:hb + H, :],
                        start=True, stop=True,
                    )
                spss[g] = sps
                hsbs[g] = None

            def stage_b2(g):
                sps = spss[g]
                ecol = ec_pool.tile([TILE_N, GROUP], f32, tag="ecol")
                nc.scalar.activation(ecol[:], sps[:], Act.Exp,
                                     bias=b2_s[:], scale=1.0)
                spss[g] = None
                ohw = oh_pool.tile([TILE_N, GROUP * SEGW], bf16, tag="ohw")
                xn = xns[g]
                oh0_v = xn[:].rearrange("p (t c) -> p t c", c=CW)[:, :, 1 + D:]
                ec_v = ecol[:].unsqueeze(2).broadcast_to([TILE_N, GROUP, SEGW])
                nc.vector.tensor_tensor(
                    ohw[:].rearrange("p (t c) -> p t c", c=SEGW),
                    oh0_v, ec_v, op=Alu.mult)
                ohws[g] = ohw

            def stage_c(g):
                nonlocal pps
                ohw = ohws[g]
                xn = xns[g]
                for j in range(GROUP):
                    tt = g * GROUP + j
                    bt, ti = divmod(tt, K)
                    if bt >= BLOCKS_PER_CORE:
                        break        # tail padding tiles (all-zero)
                    if ti == 0:
                        pps = pps_pool.tile([TILE_N, 129], f32, tag="pps")
                    a = ti // T32
                    nc.tensor.matmul(
                        pps[a * SEGW:(a + 1) * SEGW, :],
                        ohw[:, j * SEGW:(j + 1) * SEGW],
                        xn[:, j * CW:j * CW + 129],
                        start=(ti % T32 == 0),
                        stop=(ti % T32 == T32 - 1),
                        tile_position=(0, a * SEGW),
                        skip_group_check=True,
                    )
                    if ti == K - 1:
                        flush(bt, pps)
                ohws[g] = None
                xns[g] = None

            for g in range(NG + 3):
                if g < NG:
                    stage_a(g)
                if 1 <= g <= NG:
                    stage_b(g - 1)
                if 2 <= g <= NG + 1:
                    stage_b2(g - 2)
                if 3 <= g <= NG + 2:
                    stage_c(g - 3)

    nc.compile()
    return nc


def kernel(**inputs):
    x = np.asarray(inputs["x"], dtype=np.float32)
    batch = np.asarray(inputs["batch"]).astype(np.int64)
    W1 = np.asarray(inputs["W1"], dtype=np.float32)
    b1 = np.asarray(inputs["b1"], dtype=np.float32)
    W2 = np.asarray(inputs["W2"], dtype=np.float32)
    b2 = np.asarray(inputs["b2"], dtype=np.float32)

    in_maps, T32, b2f = _host_prep(x, batch, W1, b1, W2, b2)
    nc = _build(T32, b2f)

    from concourse.bass_utils import run_bass_kernel_spmd
    res = run_bass_kernel_spmd(nc, in_maps, core_ids=list(range(NCORES)))
    out = np.concatenate([r["out"] for r in res.results], axis=0)
    return out.astype(np.float32)


# revision 27
# speedup vs baseline: 1.2637x; 1.2637x over previous
"""AttentionPooling (segment softmax + weighted segment-sum) Trainium2 kernel.

Algorithm (reference without explicit seg_max subtraction — scores are tiny,
|s| < ~3, so exp() is numerically safe unshifted and softmax is
shift-invariant):

    s_i   = W2^T lrelu(W1^T x_i + b1) + b2          (per node)
    e_i   = exp(s_i)
    out_g = (sum_{i in g} e_i x_i) / (sum_{i in g} e_i + 1e-16)

Sharding: 16384 segments -> 8 cores x 16 blocks x 128 segments. batch is
sorted, so each (core, block) owns a contiguous node range. Each block is
further split into 4 sub-blocks of 32 segments; the host pads every
sub-block to the same number T32 of 128-node tiles (SPMD-uniform across
cores), so every tile's nodes fall inside one 32-segment window whose index
is a compile-time constant.

Software pipeline over 8-tile groups, 4 stages staggered one group apart so
every cross-engine dependency has >= 1 full group of slack:
  A(g):   DMA xT (fp8, score path) + xn (bf16 [ones | x | onehot0] per
          tile); mm1 (stacked lrelu trick, signs folded into
          w1c = [W1 | -W1]); relu split by columns: half ACT (bias=b1c),
          half DVE (tensor_scalar add+max).
  B(g-1): per-tile score matmul (w2c rhs) -> [128,8] psum column tile.
  B2(g-2): ONE ACT Exp (bias=b2) -> e-columns f32; ONE DVE tensor_tensor
          multiplies all 8 one-hots by their e-columns (broadcast AP).
  C(g-3): M=32 col-tiled pooling matmul accumulates [sum e | sum e*x] into
          the block's [128,129] PSUM at partition base 32*window; flush per
          block: +1e-16, reciprocal, scale, DMA out.
"""

import os
import numpy as np
import ml_dtypes

N = 2_000_000
D = 128
H = 64
G = 16384
NEG_SLOPE = 0.01
NCORES = 8
SEGS_PER_CORE = G // NCORES          # 2048
SEGS_PER_BLOCK = 128
BLOCKS_PER_CORE = SEGS_PER_CORE // SEGS_PER_BLOCK   # 16
SUBS_PER_BLOCK = 4                   # 32-segment windows
SEGW = 32
TILE_N = 128
GROUP = 8                            # tiles per group
CW = D + 1 + SEGW                    # tile width: [ones | x | onehot]
ACT_COLS = 512                       # relu columns on ACT; rest on DVE

_bf16 = ml_dtypes.bfloat16
_fp8 = ml_dtypes.float8_e4m3


def _host_prep(x, batch, W1, b1, W2, b2):
    bounds = np.searchsorted(batch, np.arange(0, G + 1, SEGW))
    cnts = np.diff(bounds)                       # nodes per 32-seg sub-block
    T32 = int(np.max((cnts + TILE_N - 1) // TILE_N))   # tiles per sub-block
    K = SUBS_PER_BLOCK * T32                     # tiles per block
    NT = BLOCKS_PER_CORE * K                     # tiles per core
    NG = (NT + GROUP - 1) // GROUP
    NTP = NG * GROUP                             # padded tile count

    iota = np.arange(SEGW, dtype=np.float32)
    in_maps = []
    for c in range(NCORES):
        xn = np.zeros((NTP, TILE_N, CW), dtype=np.float32)  # [tile, node, 1+D+32]
        for b in range(BLOCKS_PER_CORE):
            for q in range(SUBS_PER_BLOCK):
                sb = (c * BLOCKS_PER_CORE + b) * SUBS_PER_BLOCK + q
                lo, hi = int(bounds[sb]), int(bounds[sb + 1])
                n = hi - lo
                tb = b * K + q * T32
                blk = np.zeros((T32 * TILE_N, D), dtype=np.float32)
                blk[:n] = x[lo:hi]
                bl = np.full((T32 * TILE_N,), -1.0, dtype=np.float32)
                bl[:n] = (batch[lo:hi] - sb * SEGW).astype(np.float32)
                oh = (bl[:, None] == iota[None, :]).astype(np.float32)
                xn[tb:tb + T32, :, 1:1 + D] = blk.reshape(T32, TILE_N, D)
                xn[tb:tb + T32, :, 0] = 1.0
                xn[tb:tb + T32, :, 1 + D:] = oh.reshape(T32, TILE_N, SEGW)
        xn16 = xn.astype(_bf16)
        # xT groups (fp8): [NG*D, GROUP*TILE_N]
        xT = np.ascontiguousarray(
            xn[:, :, 1:1 + D].transpose(0, 2, 1)
            .reshape(NG, GROUP, D, TILE_N).transpose(0, 2, 1, 3)
            .reshape(NG * D, GROUP * TILE_N)).astype(_fp8)
        # xn groups: [NG*TILE_N, GROUP*CW]
        xng = np.ascontiguousarray(
            xn16.reshape(NG, GROUP, TILE_N, CW).transpose(0, 2, 1, 3)
            .reshape(NG * TILE_N, GROUP * CW))
        in_maps.append({"xT": xT, "xn": xng})

    consts = {
        "w1": np.ascontiguousarray(
            np.concatenate([W1, -W1], axis=1).astype(_fp8)),               # [128, 128]
        "w2c": np.ascontiguousarray(
            np.concatenate([W2, -NEG_SLOPE * W2], axis=0).astype(_bf16)),  # [128, 1]
        "b1c": np.ascontiguousarray(
            np.concatenate([b1, -b1])[:, None].astype(np.float32)),        # [128, 1]
        "b2c": np.full((TILE_N, 1), float(b2[0]), dtype=np.float32),       # [128, 1]
    }
    for m in in_maps:
        m.update(consts)
    return in_maps, T32, float(b2[0])


def _build(T32, b2f):
    import concourse.bass as bass
    import concourse.bacc as bacc
    import concourse.mybir as mybir
    from concourse.tile import TileContext

    dt = mybir.dt
    f32, bf16, fp8 = dt.float32, dt.bfloat16, dt.float8e4
    Alu = mybir.AluOpType
    Act = mybir.ActivationFunctionType

    K = SUBS_PER_BLOCK * T32
    NT = BLOCKS_PER_CORE * K
    NG = (NT + GROUP - 1) // GROUP
    NTP = NG * GROUP

    nc = bacc.Bacc("TRN2", target_bir_lowering=False)
    xT_d = nc.dram_tensor("xT", [NG * D, GROUP * TILE_N], fp8, kind="ExternalInput")
    xn_d = nc.dram_tensor("xn", [NG * TILE_N, GROUP * CW], bf16, kind="ExternalInput")
    w1_d = nc.dram_tensor("w1", [D, 2 * H], fp8, kind="ExternalInput")
    w2_d = nc.dram_tensor("w2c", [TILE_N, 1], bf16, kind="ExternalInput")
    b1_d = nc.dram_tensor("b1c", [TILE_N, 1], f32, kind="ExternalInput")
    b2_d = nc.dram_tensor("b2c", [TILE_N, 1], f32, kind="ExternalInput")
    out_d = nc.dram_tensor("out", [SEGS_PER_CORE, D], f32, kind="ExternalOutput")

    xT_v = xT_d[:].rearrange("(g p) c -> g p c", p=D)
    xn_v = xn_d[:].rearrange("(g p) c -> g p c", p=TILE_N)

    with TileContext(nc) as tc:
        import contextlib
        ctx = contextlib.ExitStack()
        with ctx:
            cpool = ctx.enter_context(tc.tile_pool(name="consts", bufs=1))
            w1_s = cpool.tile([D, 2 * H], fp8, tag="w1")
            w2_s = cpool.tile([TILE_N, 1], bf16, tag="w2")
            b1_s = cpool.tile([TILE_N, 1], f32, tag="b1")
            b2_s = cpool.tile([TILE_N, 1], f32, tag="b2")
            nc.sync.dma_start(w1_s[:], w1_d[:])
            nc.sync.dma_start(w2_s[:], w2_d[:])
            nc.sync.dma_start(b1_s[:], b1_d[:])
            nc.sync.dma_start(b2_s[:], b2_d[:])

            xg_pool = ctx.enter_context(tc.tile_pool(name="xg", bufs=8))
            xn_pool = ctx.enter_context(tc.tile_pool(name="xnp", bufs=8))
            hsb_pool = ctx.enter_context(tc.tile_pool(name="hsb", bufs=4))
            ec_pool = ctx.enter_context(tc.tile_pool(name="ec", bufs=3))
            oh_pool = ctx.enter_context(tc.tile_pool(name="oh", bufs=3))
            ob_pool = ctx.enter_context(tc.tile_pool(name="ob", bufs=2))
            dn_pool = ctx.enter_context(tc.tile_pool(name="dn", bufs=2))

            hps_pool = ctx.enter_context(tc.tile_pool(name="hps", bufs=2, space="PSUM"))
            sps_pool = ctx.enter_context(tc.tile_pool(name="sps", bufs=2, space="PSUM"))
            pps_pool = ctx.enter_context(tc.tile_pool(name="pps", bufs=2, space="PSUM"))

            def flush(bt, pps):
                dn = dn_pool.tile([TILE_N, 1], f32, tag="dn")
                nc.vector.tensor_scalar(dn[:], pps[:, 0:1], 1e-16, None, op0=Alu.add)
                rc = dn_pool.tile([TILE_N, 1], f32, tag="rc")
                nc.vector.reciprocal(rc[:], dn[:])
                ob = ob_pool.tile([TILE_N, D], f32, tag="ob")
                nc.vector.tensor_scalar(ob[:], pps[:, 1:129], rc[:], None, op0=Alu.mult)
                nc.sync.dma_start(out_d[bt * TILE_N:(bt + 1) * TILE_N, :], ob[:])

            repeat = int(os.environ.get("BASSK_REPEAT", "1"))
            if repeat > 1:
                rloop = ctx.enter_context(tc.For_i(0, repeat, 1))

            xns = [None] * NG
            hsbs = [None] * NG
            spss = [None] * NG
            ohws = [None] * NG
            pps = None

            def stage_a(g):
                xg = xg_pool.tile([D, GROUP * TILE_N], fp8, tag="xg")
                nc.sync.dma_start(xg[:], xT_v[g, :, :])
                xn = xn_pool.tile([TILE_N, GROUP * CW], bf16, tag="xn")
                nc.sync.dma_start(xn[:], xn_v[g, :, :])
                xns[g] = xn
                hps = hps_pool.tile([TILE_N, 1024], f32, tag="hps")
                for half in range(2):
                    nc.tensor.matmul(
                        hps[:, half * 512:(half + 1) * 512],
                        w1_s[:],
                        xg[:, half * 512:(half + 1) * 512],
                        start=True, stop=True,
                    )
                hsb = hsb_pool.tile([TILE_N, 1024], bf16, tag="hsb")
                nc.scalar.activation(hsb[:, 0:ACT_COLS], hps[:, 0:ACT_COLS],
                                     Act.Relu, bias=b1_s[:], scale=1.0)
                nc.vector.tensor_scalar(hsb[:, ACT_COLS:1024],
                                        hps[:, ACT_COLS:1024],
                                        b1_s[:], 0.0, op0=Alu.add, op1=Alu.max)
                hsbs[g] = hsb

            def stage_b(g):
                hsb = hsbs[g]
                sps = sps_pool.tile([TILE_N, GROUP], f32, tag="sps")
                for j in range(GROUP):
                    nc.tensor.matmul(
                        sps[:, j:j + 1],
                        hsb[:, j * TILE_N:(j + 1) * TILE_N],
                        w2_s[:],
                        start=True, stop=True,
                    )
                spss[g] = sps
                hsbs[g] = None

            def stage_b2(g):
                sps = spss[g]
                ecol = ec_pool.tile([TILE_N, GROUP], f32, tag="ecol")
                nc.scalar.activation(ecol[:], sps[:], Act.Exp,
                                     bias=b2_s[:], scale=1.0)
                spss[g] = None
                ohw = oh_pool.tile([TILE_N, GROUP * SEGW], bf16, tag="ohw")
                xn = xns[g]
                oh0_v = xn[:].rearrange("p (t c) -> p t c", c=CW)[:, :, 1 + D:]
                ec_v = ecol[:].unsqueeze(2).broadcast_to([TILE_N, GROUP, SEGW])
                nc.vector.tensor_tensor(
                    ohw[:].rearrange("p (t c) -> p t c", c=SEGW),
                    oh0_v, ec_v, op=Alu.mult)
                ohws[g] = ohw

            def stage_c(g):
                nonlocal pps
                ohw = ohws[g]
                xn = xns[g]
                for j in range(GROUP):
                    tt = g * GROUP + j
                    bt, ti = divmod(tt, K)
                    if bt >= BLOCKS_PER_CORE:
                        break        # tail padding tiles (all-zero)
                    if ti == 0:
                        pps = pps_pool.tile([TILE_N, 129], f32, tag="pps")
                    a = ti // T32
                    nc.tensor.matmul(
                        pps[a * SEGW:(a + 1) * SEGW, :],
                        ohw[:, j * SEGW:(j + 1) * SEGW],
                        xn[:, j * CW:j * CW + 129],
                        start=(ti % T32 == 0),
                        stop=(ti % T32 == T32 - 1),
                        tile_position=(0, a * SEGW),
                        skip_group_check=True,
                    )
                    if ti == K - 1:
                        flush(bt, pps)
                ohws[g] = None
                xns[g] = None

            for g in range(NG + 3):
                if g < NG:
                    stage_a(g)
                if 1 <= g <= NG:
                    stage_b(g - 1)
                if 2 <= g <= NG + 1:
                    stage_b2(g - 2)
                if 3 <= g <= NG + 2:
                    stage_c(g - 3)

    nc.compile()
    return nc


def kernel(**inputs):
    x = np.asarray(inputs["x"], dtype=np.float32)
    batch = np.asarray(inputs["batch"]).astype(np.int64)
    W1 = np.asarray(inputs["W1"], dtype=np.float32)
    b1 = np.asarray(inputs["b1"], dtype=np.float32)
    W2 = np.asarray(inputs["W2"], dtype=np.float32)
    b2 = np.asarray(inputs["b2"], dtype=np.float32)

    in_maps, T32, b2f = _host_prep(x, batch, W1, b1, W2, b2)
    nc = _build(T32, b2f)

    from concourse.bass_utils import run_bass_kernel_spmd
    res = run_bass_kernel_spmd(nc, in_maps, core_ids=list(range(NCORES)))
    out = np.concatenate([r["out"] for r in res.results], axis=0)
    return out.astype(np.float32)


# revision 28
# speedup vs baseline: 1.2904x; 1.0212x over previous
"""AttentionPooling (segment softmax + weighted segment-sum) Trainium2 kernel.

Algorithm (reference without explicit seg_max subtraction — scores are tiny,
|s| < ~3, so exp() is numerically safe unshifted and softmax is
shift-invariant):

    s_i   = W2^T lrelu(W1^T x_i + b1) + b2          (per node)
    e_i   = exp(s_i)
    out_g = (sum_{i in g} e_i x_i) / (sum_{i in g} e_i + 1e-16)

Sharding: 16384 segments -> 8 cores x 16 blocks x 128 segments. batch is
sorted, so each (core, block) owns a contiguous node range. Each block is
further split into 4 sub-blocks of 32 segments; the host pads every
sub-block to the same number T32 of 128-node tiles (SPMD-uniform across
cores), so every tile's nodes fall inside one 32-segment window whose index
is a compile-time constant.

Software pipeline over 8-tile groups, 4 stages staggered one group apart so
every cross-engine dependency has >= 1 full group of slack:
  A(g):   DMA xT (fp8, score path) + xn (bf16 [ones | x | onehot0] per
          tile); mm1 (stacked lrelu trick, signs folded into
          w1c = [W1 | -W1]); relu split by columns: half ACT (bias=b1c),
          half DVE (tensor_scalar add+max).
  B(g-1): per-tile score matmul (w2c rhs) -> [128,8] psum column tile.
  B2(g-2): ONE ACT Exp (bias=b2) -> e-columns f32; ONE DVE tensor_tensor
          multiplies all 8 one-hots by their e-columns (broadcast AP).
  C(g-3): M=32 col-tiled pooling matmul accumulates [sum e | sum e*x] into
          the block's [128,129] PSUM at partition base 32*window; flush per
          block: +1e-16, reciprocal, scale, DMA out.
"""

import os
import numpy as np
import ml_dtypes

N = 2_000_000
D = 128
H = 64
G = 16384
NEG_SLOPE = 0.01
NCORES = 8
SEGS_PER_CORE = G // NCORES          # 2048
SEGS_PER_BLOCK = 128
BLOCKS_PER_CORE = SEGS_PER_CORE // SEGS_PER_BLOCK   # 16
SUBS_PER_BLOCK = 4                   # 32-segment windows
SEGW = 32
TILE_N = 128
GROUP = 8                            # tiles per group
CW = D + 1 + SEGW                    # tile width: [ones | x | onehot]
ACT_COLS = 512                       # relu columns on ACT; rest on DVE

_bf16 = ml_dtypes.bfloat16
_fp8 = ml_dtypes.float8_e4m3


def _host_prep(x, batch, W1, b1, W2, b2):
    bounds = np.searchsorted(batch, np.arange(0, G + 1, SEGW))
    cnts = np.diff(bounds)                       # nodes per 32-seg sub-block
    T32 = int(np.max((cnts + TILE_N - 1) // TILE_N))   # tiles per sub-block
    K = SUBS_PER_BLOCK * T32                     # tiles per block
    NT = BLOCKS_PER_CORE * K                     # tiles per core
    NG = (NT + GROUP - 1) // GROUP
    NTP = NG * GROUP                             # padded tile count

    iota = np.arange(SEGW, dtype=np.float32)
    in_maps = []
    for c in range(NCORES):
        xn = np.zeros((NTP, TILE_N, CW), dtype=np.float32)  # [tile, node, 1+D+32]
        for b in range(BLOCKS_PER_CORE):
            for q in range(SUBS_PER_BLOCK):
                sb = (c * BLOCKS_PER_CORE + b) * SUBS_PER_BLOCK + q
                lo, hi = int(bounds[sb]), int(bounds[sb + 1])
                n = hi - lo
                tb = b * K + q * T32
                blk = np.zeros((T32 * TILE_N, D), dtype=np.float32)
                blk[:n] = x[lo:hi]
                bl = np.full((T32 * TILE_N,), -1.0, dtype=np.float32)
                bl[:n] = (batch[lo:hi] - sb * SEGW).astype(np.float32)
                oh = (bl[:, None] == iota[None, :]).astype(np.float32)
                xn[tb:tb + T32, :, 1:1 + D] = blk.reshape(T32, TILE_N, D)
                xn[tb:tb + T32, :, 0] = 1.0
                xn[tb:tb + T32, :, 1 + D:] = oh.reshape(T32, TILE_N, SEGW)
        xn16 = xn.astype(_bf16)
        # xT groups (fp8): [NG*D, GROUP*TILE_N]
        xT = np.ascontiguousarray(
            xn[:, :, 1:1 + D].transpose(0, 2, 1)
            .reshape(NG, GROUP, D, TILE_N).transpose(0, 2, 1, 3)
            .reshape(NG * D, GROUP * TILE_N)).astype(_fp8)
        # xn groups: [NG*TILE_N, GROUP*CW]
        xng = np.ascontiguousarray(
            xn16.reshape(NG, GROUP, TILE_N, CW).transpose(0, 2, 1, 3)
            .reshape(NG * TILE_N, GROUP * CW))
        in_maps.append({"xT": xT, "xn": xng})

    consts = {
        "w1": np.ascontiguousarray(
            np.concatenate([W1, -W1], axis=1).astype(_fp8)),               # [128, 128]
        "w2c": np.ascontiguousarray(
            np.concatenate([W2, -NEG_SLOPE * W2], axis=0).astype(_bf16)),  # [128, 1]
        "b1c": np.ascontiguousarray(
            np.concatenate([b1, -b1])[:, None].astype(np.float32)),        # [128, 1]
        "b2c": np.full((TILE_N, 1), float(b2[0]), dtype=np.float32),       # [128, 1]
    }
    for m in in_maps:
        m.update(consts)
    return in_maps, T32, float(b2[0])


def _build(T32, b2f):
    import concourse.bass as bass
    import concourse.bacc as bacc
    import concourse.mybir as mybir
    from concourse.tile import TileContext

    dt = mybir.dt
    f32, bf16, fp8 = dt.float32, dt.bfloat16, dt.float8e4
    Alu = mybir.AluOpType
    Act = mybir.ActivationFunctionType

    K = SUBS_PER_BLOCK * T32
    NT = BLOCKS_PER_CORE * K
    NG = (NT + GROUP - 1) // GROUP
    NTP = NG * GROUP

    nc = bacc.Bacc("TRN2", target_bir_lowering=False)
    xT_d = nc.dram_tensor("xT", [NG * D, GROUP * TILE_N], fp8, kind="ExternalInput")
    xn_d = nc.dram_tensor("xn", [NG * TILE_N, GROUP * CW], bf16, kind="ExternalInput")
    w1_d = nc.dram_tensor("w1", [D, 2 * H], fp8, kind="ExternalInput")
    w2_d = nc.dram_tensor("w2c", [TILE_N, 1], bf16, kind="ExternalInput")
    b1_d = nc.dram_tensor("b1c", [TILE_N, 1], f32, kind="ExternalInput")
    b2_d = nc.dram_tensor("b2c", [TILE_N, 1], f32, kind="ExternalInput")
    out_d = nc.dram_tensor("out", [SEGS_PER_CORE, D], f32, kind="ExternalOutput")

    xT_v = xT_d[:].rearrange("(g p) c -> g p c", p=D)
    xn_v = xn_d[:].rearrange("(g p) c -> g p c", p=TILE_N)

    with TileContext(nc) as tc:
        import contextlib
        ctx = contextlib.ExitStack()
        with ctx:
            cpool = ctx.enter_context(tc.tile_pool(name="consts", bufs=1))
            w1_s = cpool.tile([D, 2 * H], fp8, tag="w1")
            w2_s = cpool.tile([TILE_N, 1], bf16, tag="w2")
            b1_s = cpool.tile([TILE_N, 1], f32, tag="b1")
            b2_s = cpool.tile([TILE_N, 1], f32, tag="b2")
            nc.sync.dma_start(w1_s[:], w1_d[:])
            nc.sync.dma_start(w2_s[:], w2_d[:])
            nc.sync.dma_start(b1_s[:], b1_d[:])
            nc.sync.dma_start(b2_s[:], b2_d[:])

            xg_pool = ctx.enter_context(tc.tile_pool(name="xg", bufs=10))
            xn_pool = ctx.enter_context(tc.tile_pool(name="xnp", bufs=10))
            hsb_pool = ctx.enter_context(tc.tile_pool(name="hsb", bufs=6))
            ec_pool = ctx.enter_context(tc.tile_pool(name="ec", bufs=4))
            oh_pool = ctx.enter_context(tc.tile_pool(name="oh", bufs=4))
            ob_pool = ctx.enter_context(tc.tile_pool(name="ob", bufs=3))
            dn_pool = ctx.enter_context(tc.tile_pool(name="dn", bufs=3))

            hps_pool = ctx.enter_context(tc.tile_pool(name="hps", bufs=2, space="PSUM"))
            sps_pool = ctx.enter_context(tc.tile_pool(name="sps", bufs=2, space="PSUM"))
            pps_pool = ctx.enter_context(tc.tile_pool(name="pps", bufs=2, space="PSUM"))

            def flush(bt, pps):
                dn = dn_pool.tile([TILE_N, 1], f32, tag="dn")
                nc.vector.tensor_scalar(dn[:], pps[:, 0:1], 1e-16, None, op0=Alu.add)
                rc = dn_pool.tile([TILE_N, 1], f32, tag="rc")
                nc.vector.reciprocal(rc[:], dn[:])
                ob = ob_pool.tile([TILE_N, D], f32, tag="ob")
                nc.vector.tensor_scalar(ob[:], pps[:, 1:129], rc[:], None, op0=Alu.mult)
                nc.sync.dma_start(out_d[bt * TILE_N:(bt + 1) * TILE_N, :], ob[:])

            repeat = int(os.environ.get("BASSK_REPEAT", "1"))
            if repeat > 1:
                rloop = ctx.enter_context(tc.For_i(0, repeat, 1))

            xns = [None] * NG
            hsbs = [None] * NG
            spss = [None] * NG
            ohws = [None] * NG
            pps = None

            def stage_a(g):
                xg = xg_pool.tile([D, GROUP * TILE_N], fp8, tag="xg")
                nc.sync.dma_start(xg[:], xT_v[g, :, :])
                xn = xn_pool.tile([TILE_N, GROUP * CW], bf16, tag="xn")
                nc.sync.dma_start(xn[:], xn_v[g, :, :])
                xns[g] = xn
                hps = hps_pool.tile([TILE_N, 1024], f32, tag="hps")
                for half in range(2):
                    nc.tensor.matmul(
                        hps[:, half * 512:(half + 1) * 512],
                        w1_s[:],
                        xg[:, half * 512:(half + 1) * 512],
                        start=True, stop=True,
                    )
                hsb = hsb_pool.tile([TILE_N, 1024], bf16, tag="hsb")
                nc.scalar.activation(hsb[:, 0:ACT_COLS], hps[:, 0:ACT_COLS],
                                     Act.Relu, bias=b1_s[:], scale=1.0)
                nc.vector.tensor_scalar(hsb[:, ACT_COLS:1024],
                                        hps[:, ACT_COLS:1024],
                                        b1_s[:], 0.0, op0=Alu.add, op1=Alu.max)
                hsbs[g] = hsb

            def stage_b(g):
                hsb = hsbs[g]
                sps = sps_pool.tile([TILE_N, GROUP], f32, tag="sps")
                for j in range(GROUP):
                    nc.tensor.matmul(
                        sps[:, j:j + 1],
                        hsb[:, j * TILE_N:(j + 1) * TILE_N],
                        w2_s[:],
                        start=True, stop=True,
                    )
                spss[g] = sps
                hsbs[g] = None

            def stage_b2(g):
                sps = spss[g]
                ecol = ec_pool.tile([TILE_N, GROUP], f32, tag="ecol")
                nc.scalar.activation(ecol[:], sps[:], Act.Exp,
                                     bias=b2_s[:], scale=1.0)
                spss[g] = None
                ohw = oh_pool.tile([TILE_N, GROUP * SEGW], bf16, tag="ohw")
                xn = xns[g]
                oh0_v = xn[:].rearrange("p (t c) -> p t c", c=CW)[:, :, 1 + D:]
                ec_v = ecol[:].unsqueeze(2).broadcast_to([TILE_N, GROUP, SEGW])
                nc.vector.tensor_tensor(
                    ohw[:].rearrange("p (t c) -> p t c", c=SEGW),
                    oh0_v, ec_v, op=Alu.mult)
                ohws[g] = ohw

            def stage_c(g):
                nonlocal pps
                ohw = ohws[g]
                xn = xns[g]
                for j in range(GROUP):
                    tt = g * GROUP + j
                    bt, ti = divmod(tt, K)
                    if bt >= BLOCKS_PER_CORE:
                        break        # tail padding tiles (all-zero)
                    if ti == 0:
                        pps = pps_pool.tile([TILE_N, 129], f32, tag="pps")
                    a = ti // T32
                    nc.tensor.matmul(
                        pps[a * SEGW:(a + 1) * SEGW, :],
                        ohw[:, j * SEGW:(j + 1) * SEGW],
                        xn[:, j * CW:j * CW + 129],
                        start=(ti % T32 == 0),
                        stop=(ti % T32 == T32 - 1),
                        tile_position=(0, a * SEGW),
                        skip_group_check=True,
                    )
                    if ti == K - 1:
                        flush(bt, pps)
                ohws[g] = None
                xns[g] = None

            for g in range(NG + 3):
                if g < NG:
                    stage_a(g)
                if 1 <= g <= NG:
                    stage_b(g - 1)
                if 2 <= g <= NG + 1:
                    stage_b2(g - 2)
                if 3 <= g <= NG + 2:
                    stage_c(g - 3)

    nc.compile()
    return nc


def kernel(**inputs):
    x = np.asarray(inputs["x"], dtype=np.float32)
    batch = np.asarray(inputs["batch"]).astype(np.int64)
    W1 = np.asarray(inputs["W1"], dtype=np.float32)
    b1 = np.asarray(inputs["b1"], dtype=np.float32)
    W2 = np.asarray(inputs["W2"], dtype=np.float32)
    b2 = np.asarray(inputs["b2"], dtype=np.float32)

    in_maps, T32, b2f = _host_prep(x, batch, W1, b1, W2, b2)
    nc = _build(T32, b2f)

    from concourse.bass_utils import run_bass_kernel_spmd
    res = run_bass_kernel_spmd(nc, in_maps, core_ids=list(range(NCORES)))
    out = np.concatenate([r["out"] for r in res.results], axis=0)
    return out.astype(np.float32)
